# revision 16
# baseline (speedup 1.0000x reference)
"""Trainium2 Bass kernel for DeepDFT Message+Receiver block.

Computes, for inputs of shape
  scalar [B,A,G,F], scalar_reciever [B,P,G,F], expansion [B,P,A,E],
  mask [B,P,A,G], edge_distance [B,P,A,1], + MLP weights:

  gates = ssp(expansion @ W1e + b1e) @ W2e + b2e
  gates *= 1 - sigmoid(5*(edge_distance - (CUTOFF-1.5)))
  src = scalar @ W1n[:F]; tgt = reciever @ W1n[F:]
  nodes = ssp(src + tgt + b1n) @ W2n + b2n
  out = sum_a mask * gates * nodes          -> [B,P,G,F]

Sharding: probe axis P across 8 cores. Within a core the atom axis is
split into 4 quarters mapped onto the 4 32-partition groups (features on
partitions), columns = (b, p, a_local).

Key identities (exact):
  ssp(x) = softplus(x) - log2 = ln(exp(x - log2) + 0.5)
  exp(src+tgt+b1n-log2) = exp(src+b1n-log2) * exp(tgt)   (tiny factors)
so each ssp costs one Ln pass on ACT; the gates path needs one extra Exp
pass; all biases fold into the exponent shifts (b1e/b1n/b2e/b2n are
handled generally below).
"""
import sys, os
if "/opt/trn_rl_repo" not in sys.path:
    sys.path.insert(0, "/opt/trn_rl_repo")
os.environ.setdefault("JAX_PLATFORMS", "cpu,axon")

import numpy as np
import ml_dtypes

B, P, A, G, F, E = 2, 4096, 96, 1, 32, 20
NCORES = 8
PLOC = P // NCORES          # 512 probes per core
NGRP = 4                    # atom quarter groups
AL = A // NGRP              # 24 atoms per group
NPAIR = B * PLOC            # 1024 (b,p) pairs per core
NCOLS = NPAIR * AL          # 24576 cols per group
CH = 384                    # chunk = 16 probes * 24 atoms
CPP = CH // AL              # 16 probes per chunk
NCH = NCOLS // CH           # 64 chunks
MACC = 2                    # chunks per psum macro
NMAC = NCH // MACC          # 32 macros
SGM = 4                     # macros per tree-stage flush (4*768 = 3072 cols = 128 p)
LOG2 = 0.6931471805599453
CUTOFF = 5.0

_CACHE = {}

# Opcodes whose sem updates are executed by DMA hardware (riding the
# descriptor) rather than the issuing sequencer — their updates must not be
# moved onto a NOP.
_DMA_OPCODES = ("TensorLoad", "TensorSave", "TensorCopy", "Dge", "DMA")


def _fix_bir_json(raw: bytes) -> bytes:
    """This walrus build accepts at most ONE sem wait (and one update) per
    instruction (NEURON_ISA_TPB_EVENTS has a single wait/update slot).
    Split excess waits onto preceding same-engine NOPs (sequencer order
    makes this equivalent) and excess updates onto trailing NOPs."""
    import json
    m = json.loads(raw)
    ctr = [0]

    def mknop(engine, wait=None, upd=None):
        ctr[0] += 1
        return {
            "engine": engine, "ins": [], "outs": [],
            "name": f"I-wsplit-{ctr[0]}", "opcode": "NoOp",
            "sync_info": {
                "on_wait": [wait] if wait else [],
                "on_update": [upd] if upd else [],
            },
        }

    for fn in m["functions"]:
        for bb in fn["blocks"]:
            newl = []
            for inst in bb["instructions"]:
                si = inst.get("sync_info")
                pre, post = [], []
                if si:
                    w = si.get("on_wait") or []
                    if len(w) > 1:
                        for x in w[:-1]:
                            pre.append(mknop(inst["engine"], wait=x))
                        si["on_wait"] = [w[-1]]
                    u = si.get("on_update") or []
                    if len(u) > 1:
                        op = str(inst.get("opcode", ""))
                        assert not any(d in op for d in _DMA_OPCODES), (
                            f"multi-update DMA instruction {inst.get('name')}"
                        )
                        for x in u[1:]:
                            post.append(mknop(inst["engine"], upd=x))
                        si["on_update"] = [u[0]]
                newl.extend(pre)
                newl.append(inst)
                newl.extend(post)
            bb["instructions"] = newl
    return json.dumps(m).encode()


def _build_bass():
    import concourse.bass as bass
    import concourse.mybir as mybir
    from tile_fix_embedded import SplitDrainTileContext

    f32 = mybir.dt.float32
    bf16 = mybir.dt.bfloat16
    AF = mybir.ActivationFunctionType
    OP = mybir.AluOpType

    nc = bass.Bass(num_devices=NCORES)

    # ---- DRAM I/O ----
    d_expT = nc.dram_tensor("expT", [NGRP * E, NCOLS], f32, kind="ExternalInput")
    d_maskq = nc.dram_tensor("maskq", [128, NCOLS // 32], f32, kind="ExternalInput")
    d_edgeq = nc.dram_tensor("edgeq", [128, NCOLS // 32], f32, kind="ExternalInput")
    d_recvT = nc.dram_tensor("recvT", [F, NPAIR], f32, kind="ExternalInput")
    d_srcT = nc.dram_tensor("srcT", [F, B * A], f32, kind="ExternalInput")
    d_bdW1e = nc.dram_tensor("bdW1e", [NGRP * E, 128], f32, kind="ExternalInput")
    d_bdW2e = nc.dram_tensor("bdW2e", [128, 128], f32, kind="ExternalInput")
    d_bdW2n = nc.dram_tensor("bdW2n", [128, 128], f32, kind="ExternalInput")
    d_bdSum = nc.dram_tensor("bdSum", [128, F], f32, kind="ExternalInput")
    d_wsT = nc.dram_tensor("wsT", [F, F], f32, kind="ExternalInput")
    d_wtT = nc.dram_tensor("wtT", [F, F], f32, kind="ExternalInput")
    d_bEx = nc.dram_tensor("bEx", [128, 1], f32, kind="ExternalInput")    # b1e - log2 (x4)
    d_bEs = nc.dram_tensor("bEs", [F, 1], f32, kind="ExternalInput")      # b1n - log2
    # Ln scale/bias folds: act1 = Ln(E1*e^c + 0.5*e^c) = ssp(y1)+c, c = W2e^-T b2e
    d_lnSG = nc.dram_tensor("lnSG", [128, 1], f32, kind="ExternalInput")
    d_lnBG = nc.dram_tensor("lnBG", [128, 1], f32, kind="ExternalInput")
    d_lnSH = nc.dram_tensor("lnSH", [128, 1], f32, kind="ExternalInput")
    d_lnBH = nc.dram_tensor("lnBH", [128, 1], f32, kind="ExternalInput")
    d_out = nc.dram_tensor("outT", [F, NPAIR], f32, kind="ExternalOutput")

    with SplitDrainTileContext(nc) as tc:
        with (
            tc.tile_pool(name="persist", bufs=1) as pp,
            tc.tile_pool(name="work", bufs=3) as wp,
            tc.tile_pool(name="stage", bufs=2) as sp,
            tc.tile_pool(name="psA", bufs=2, space="PSUM") as psA,
            tc.tile_pool(name="psC", bufs=1, space="PSUM") as psC,
        ):
            # ---- persistent tiles ----
            w1e = pp.tile([NGRP * E, 128], f32, tag="w1e")
            w2e = pp.tile([128, 128], f32, tag="w2e")
            w2n = pp.tile([128, 128], f32, tag="w2n")
            wsum = pp.tile([128, F], f32, tag="wsum")
            ws = pp.tile([F, F], f32, tag="ws")
            wt = pp.tile([F, F], f32, tag="wt")
            bEx = pp.tile([128, 1], f32, tag="bEx")
            bEs = pp.tile([F, 1], f32, tag="bEs")
            lnSG = pp.tile([128, 1], f32, tag="lnSG")
            lnBG = pp.tile([128, 1], f32, tag="lnBG")
            lnSH = pp.tile([128, 1], f32, tag="lnSH")
            lnBH = pp.tile([128, 1], f32, tag="lnBH")
            nc.sync.dma_start(out=w1e[:], in_=d_bdW1e[:])
            nc.sync.dma_start(out=w2e[:], in_=d_bdW2e[:])
            nc.sync.dma_start(out=w2n[:], in_=d_bdW2n[:])
            nc.sync.dma_start(out=wsum[:], in_=d_bdSum[:])
            nc.sync.dma_start(out=ws[:], in_=d_wsT[:])
            nc.sync.dma_start(out=wt[:], in_=d_wtT[:])
            nc.sync.dma_start(out=bEx[:], in_=d_bEx[:])
            nc.sync.dma_start(out=bEs[:], in_=d_bEs[:])
            nc.sync.dma_start(out=lnSG[:], in_=d_lnSG[:])
            nc.sync.dma_start(out=lnBG[:], in_=d_lnBG[:])
            nc.sync.dma_start(out=lnSH[:], in_=d_lnSH[:])
            nc.sync.dma_start(out=lnBH[:], in_=d_lnBH[:])

            # ---- s = mask * sigmoid(17.5 - 5 d): do Sigmoid FIRST (table set) ----
            mq = pp.tile([128, NCOLS // 32], f32, tag="mq")
            eq = pp.tile([128, NCOLS // 32], f32, tag="eq")
            nc.sync.dma_start(out=mq[:], in_=d_maskq[:])
            nc.sync.dma_start(out=eq[:], in_=d_edgeq[:])
            sigB = pp.tile([128, 1], f32, tag="sigB")
            nc.gpsimd.memset(sigB[:], 5.0 * (CUTOFF - 1.5))
            sig = pp.tile([128, NCOLS // 32], f32, tag="sig")
            nc.scalar.activation(sig[:], eq[:], AF.Sigmoid,
                                 bias=sigB[:, 0:1], scale=-5.0)
            sqb = pp.tile([128, NCOLS // 32], bf16, tag="sqb")
            nc.vector.tensor_mul(out=sqb[:], in0=mq[:], in1=sig[:])

            # ---- S_all [128, NCOLS] bf16: row (32i+h) holds group i's s-vector ----
            S_all = pp.tile([128, NCOLS], bf16, tag="S_all")
            KQ = NCOLS // 32  # 768
            for i in range(NGRP):
                for k in range(32):
                    nc.sync.dma_start(
                        out=S_all[32 * i : 32 * i + 1, k * KQ : (k + 1) * KQ],
                        in_=sqb[32 * i + k : 32 * i + k + 1, :],
                    )
                # log-double across the 32 partitions of the group
                rep = 1
                while rep < 32:
                    nc.sync.dma_start(
                        out=S_all[32 * i + rep : 32 * i + 2 * rep, :],
                        in_=S_all[32 * i : 32 * i + rep, :],
                    )
                    rep *= 2

            # ---- es4 [128, B*AL], et4 [128, NPAIR] (bf16, exp domain) ----
            srcT = pp.tile([F, B * A], f32, tag="srcT")
            recvT = pp.tile([F, NPAIR], f32, tag="recvT")
            nc.sync.dma_start(out=srcT[:], in_=d_srcT[:])
            nc.sync.dma_start(out=recvT[:], in_=d_recvT[:])

            ps_s = psC.tile([F, B * A], f32, tag="psG")
            nc.tensor.matmul(ps_s[:], ws[:], srcT[:], start=True, stop=True)
            es_full = pp.tile([F, B * A], bf16, tag="es_full")
            nc.scalar.activation(es_full[:], ps_s[:], AF.Exp, bias=bEs[:, 0:1])

            et_full = pp.tile([F, NPAIR], bf16, tag="et_full")
            for half in range(2):
                ps_t = psC.tile([F, 512], f32, tag="psN")
                nc.tensor.matmul(ps_t[:], wt[:], recvT[:, 512 * half : 512 * (half + 1)],
                                 start=True, stop=True)
                nc.scalar.activation(et_full[:, 512 * half : 512 * (half + 1)],
                                     ps_t[:], AF.Exp)

            es4 = pp.tile([128, B * AL], bf16, tag="es4")
            et4 = pp.tile([128, NPAIR], bf16, tag="et4")
            for i in range(NGRP):
                for b in range(B):
                    nc.sync.dma_start(
                        out=es4[32 * i : 32 * i + 32, b * AL : (b + 1) * AL],
                        in_=es_full[:, b * A + AL * i : b * A + AL * (i + 1)],
                    )
                nc.sync.dma_start(out=et4[32 * i : 32 * i + 32, :], in_=et_full[:])

            # ---- output accumulator ----
            OUT4 = pp.tile([128, NPAIR], f32, tag="OUT4")

            # ---- main loop ----
            MW = MACC * CH  # 768 macro width
            for sg in range(NMAC // SGM):  # stage groups of SGM macros
                stage = sp.tile([128, SGM * MW], bf16, tag="stage")
                for mi in range(SGM):
                    m = sg * SGM + mi
                    bidx = (m * MW) // (PLOC * AL)          # which b
                    poff = ((m * MW) % (PLOC * AL)) // AL   # probe offset in b
                    npch = MW // AL                          # 32 probes per macro

                    X = wp.tile([NGRP * E, MW], f32, tag="X")
                    nc.sync.dma_start(out=X[:], in_=d_expT[:, m * MW : (m + 1) * MW])

                    ps1 = psA.tile([128, 1024], f32, tag="ps1")
                    for c in range(MACC):
                        nc.tensor.matmul(
                            ps1[:, 512 * c : 512 * c + CH],
                            w1e[:], X[:, CH * c : CH * (c + 1)],
                            start=True, stop=True)
                    ps1v = ps1[:].rearrange("p (c w) -> p c w", c=MACC)[:, :, 0:CH]
                    E1 = wp.tile([128, MW], f32, tag="E1")
                    E1v = E1[:].rearrange("p (c w) -> p c w", c=MACC)
                    nc.scalar.activation(E1v, ps1v, AF.Exp, bias=bEx[:, 0:1])

                    act1 = wp.tile([128, MW], f32, tag="act1")
                    nc.scalar.activation(act1[:], E1[:], AF.Ln,
                                         bias=lnBG[:, 0:1], scale=lnSG[:, 0:1])

                    psG = psC.tile([128, 1024], f32, tag="psG")
                    for c in range(MACC):
                        nc.tensor.matmul(
                            psG[:, 512 * c : 512 * c + CH],
                            w2e[:], act1[:, CH * c : CH * (c + 1)],
                            start=True, stop=True)

                    # ehp = es4 * et4 (broadcast views), bf16
                    ehp = wp.tile([128, MW], bf16, tag="ehp")
                    ehpv = ehp[:].rearrange("p (q w) -> p q w", q=npch)
                    esv = es4[:, None, bidx * AL : (bidx + 1) * AL].broadcast_to(
                        [128, npch, AL])
                    etv = et4[:, bidx * PLOC + poff : bidx * PLOC + poff + npch, None
                              ].broadcast_to([128, npch, AL])
                    nc.vector.tensor_mul(out=ehpv, in0=esv, in1=etv)

                    actH = wp.tile([128, MW], f32, tag="actH")
                    nc.scalar.activation(actH[:], ehp[:], AF.Ln,
                                         bias=lnBH[:, 0:1], scale=lnSH[:, 0:1])

                    psN = psC.tile([128, 1024], f32, tag="psN")
                    for c in range(MACC):
                        nc.tensor.matmul(
                            psN[:, 512 * c : 512 * c + CH],
                            w2n[:], actH[:, CH * c : CH * (c + 1)],
                            start=True, stop=True)

                    # sq = (G * s) * N  — DVE can read only one PSUM input
                    # per op, so s (SBUF) pairs with G, then N.
                    psGv = psG[:].rearrange("p (c w) -> p c w", c=MACC)[:, :, 0:CH]
                    psNv = psN[:].rearrange("p (c w) -> p c w", c=MACC)[:, :, 0:CH]
                    Sv = S_all[:, m * MW : (m + 1) * MW].rearrange(
                        "p (c w) -> p c w", c=MACC)
                    gs = wp.tile([128, MW], bf16, tag="q")
                    gsv = gs[:].rearrange("p (c w) -> p c w", c=MACC)
                    nc.vector.tensor_mul(out=gsv, in0=psGv, in1=Sv)
                    sqv = stage[:, mi * MW : (mi + 1) * MW].rearrange(
                        "p (c w) -> p c w", c=MACC)
                    nc.vector.tensor_mul(out=sqv, in0=gsv, in1=psNv)

                # tree-reduce stage [128, SGM*MW] over a_local (24)
                NPS = SGM * MW // AL  # 128 probes
                sv = stage[:].rearrange("p (n a) -> p n a", a=AL)
                t1 = sp.tile([128, NPS * 12], bf16, tag="t1")
                t1v = t1[:].rearrange("p (n a) -> p n a", a=12)
                nc.vector.tensor_add(out=t1v, in0=sv[:, :, 0:12], in1=sv[:, :, 12:24])
                t2 = sp.tile([128, NPS * 6], bf16, tag="t2")
                t2v = t2[:].rearrange("p (n a) -> p n a", a=6)
                nc.vector.tensor_add(out=t2v, in0=t1v[:, :, 0:6], in1=t1v[:, :, 6:12])
                t3 = sp.tile([128, NPS * 3], bf16, tag="t3")
                t3v = t3[:].rearrange("p (n a) -> p n a", a=3)
                nc.vector.tensor_add(out=t3v, in0=t2v[:, :, 0:3], in1=t2v[:, :, 3:6])
                t4 = sp.tile([128, NPS], f32, tag="t4")
                t4v = t4[:].rearrange("p (n a) -> p n a", a=1)
                nc.vector.tensor_add(out=t4v, in0=t3v[:, :, 0:1], in1=t3v[:, :, 1:2])
                pbase = sg * NPS
                ov = OUT4[:, pbase : pbase + NPS].rearrange("p (n a) -> p n a", a=1)
                nc.vector.tensor_add(out=ov, in0=t4v, in1=t3v[:, :, 2:3])

            # ---- cross-group sum + writeout ----
            outsb = pp.tile([F, NPAIR], f32, tag="outsb")
            for half in range(2):
                psF = psC.tile([F, 512], f32, tag="psN")
                nc.tensor.matmul(psF[:], wsum[:],
                                 OUT4[:, 512 * half : 512 * (half + 1)],
                                 start=True, stop=True)
                nc.vector.tensor_copy(outsb[:, 512 * half : 512 * (half + 1)], psF[:])
            nc.sync.dma_start(out=d_out[:], in_=outsb[:])

    # Patch serialization: enforce the 1-wait/1-update ISA slot limit.
    import types
    _orig_tjb = nc.to_json_bytes
    _fixed = {}

    def _patched_to_json_bytes(self):
        if "b" not in _fixed:
            _fixed["b"] = _fix_bir_json(_orig_tjb())
        return _fixed["b"]

    nc.to_json_bytes = types.MethodType(_patched_to_json_bytes, nc)
    return nc


def _host_prep(inputs):
    """Host-side layout prep: slicing/transpose/padding only (plus constant
    folds on the tiny weight matrices)."""
    scalar = np.asarray(inputs["scalar"], np.float32)
    reciever = np.asarray(inputs["scalar_reciever"], np.float32)
    expansion = np.asarray(inputs["expansion"], np.float32)
    mask = np.asarray(inputs["mask"], np.float32)
    edge = np.asarray(inputs["edge_distance"], np.float32)
    W1e = np.asarray(inputs["W1e"], np.float32)
    b1e = np.asarray(inputs["b1e"], np.float32)
    W2e = np.asarray(inputs["W2e"], np.float32)
    b2e = np.asarray(inputs["b2e"], np.float32)
    W1n = np.asarray(inputs["W1n"], np.float32)
    b1n = np.asarray(inputs["b1n"], np.float32)
    W2n = np.asarray(inputs["W2n"], np.float32)
    b2n = np.asarray(inputs["b2n"], np.float32)

    bdW1e = np.zeros((NGRP * E, 128), np.float32)
    bdW2e = np.zeros((128, 128), np.float32)
    bdW2n = np.zeros((128, 128), np.float32)
    bdSum = np.zeros((128, F), np.float32)
    for i in range(NGRP):
        bdW1e[i * E : (i + 1) * E, 32 * i : 32 * i + F] = W1e
        bdW2e[32 * i : 32 * i + F, 32 * i : 32 * i + F] = W2e
        bdW2n[32 * i : 32 * i + F, 32 * i : 32 * i + F] = W2n
        bdSum[32 * i : 32 * i + F, :] = np.eye(F, dtype=np.float32)
    # act1 = Ln(E1*e^cg + 0.5*e^cg) = ssp(y1) + cg with cg = W2e^-T b2e, so
    # act1 @ W2e = ssp @ W2e + b2e exactly (same for the nodes path).
    cg = np.linalg.solve(W2e.T.astype(np.float64), b2e.astype(np.float64))
    cn = np.linalg.solve(W2n.T.astype(np.float64), b2n.astype(np.float64))
    shared = {
        "bdW1e": bdW1e, "bdW2e": bdW2e, "bdW2n": bdW2n, "bdSum": bdSum,
        "wsT": np.ascontiguousarray(W1n[:F]),
        "wtT": np.ascontiguousarray(W1n[F:]),
        "bEx": np.ascontiguousarray((np.tile(b1e, NGRP) - LOG2)[:, None]),
        "bEs": np.ascontiguousarray((b1n - LOG2)[:, None]),
        "lnSG": np.tile(np.exp(cg), NGRP).astype(np.float32)[:, None].copy(),
        "lnBG": np.tile(0.5 * np.exp(cg), NGRP).astype(np.float32)[:, None].copy(),
        "lnSH": np.tile(np.exp(cn), NGRP).astype(np.float32)[:, None].copy(),
        "lnBH": np.tile(0.5 * np.exp(cn), NGRP).astype(np.float32)[:, None].copy(),
    }
    srcT = np.ascontiguousarray(scalar[:, :, 0, :].reshape(B * A, F).T)

    in_maps = []
    for c in range(NCORES):
        psl = slice(c * PLOC, (c + 1) * PLOC)
        x = expansion[:, psl].reshape(B, PLOC, NGRP, AL, E)
        expT = np.ascontiguousarray(
            x.transpose(2, 4, 0, 1, 3).reshape(NGRP * E, NCOLS))
        mq = np.ascontiguousarray(
            mask[:, psl, :, 0].reshape(B, PLOC, NGRP, AL)
            .transpose(2, 0, 1, 3).reshape(128, NCOLS // 32))
        eq = np.ascontiguousarray(
            edge[:, psl, :, 0].reshape(B, PLOC, NGRP, AL)
            .transpose(2, 0, 1, 3).reshape(128, NCOLS // 32))
        recvT = np.ascontiguousarray(
            reciever[:, psl, 0, :].reshape(NPAIR, F).T)
        in_maps.append({
            "expT": expT, "maskq": mq, "edgeq": eq,
            "recvT": recvT, "srcT": srcT, **shared,
        })
    return in_maps


def kernel(**inputs):
    if "nc" not in _CACHE:
        _CACHE["nc"] = _build_bass()
    nc = _CACHE["nc"]
    in_maps = _host_prep(inputs)

    from concourse.bass_utils import run_bass_kernel_spmd
    trace = os.environ.get("BASS_KERNEL_TRACE", "0") == "1"
    res = run_bass_kernel_spmd(nc, in_maps, core_ids=list(range(NCORES)),
                               trace=trace)
    _CACHE["last_result"] = res

    out = np.empty((B, P, G, F), np.float32)
    for c in range(NCORES):
        outT = res.results[c]["outT"]            # [F, NPAIR]
        out[:, c * PLOC : (c + 1) * PLOC, 0, :] = outT.T.reshape(B, PLOC, F)
    return out


# --- embedded TileContext fix (kernel.py must be self-contained) ---
import types as _types

_tile_fix_src = '''
import concourse.mybir as mybir
from concourse.tile import TileContext

MAX_WAITS = 1


def _split_instruction_waits(nc, drain_inst):
    si = drain_inst.ins.sync_info
    if si is None:
        return
    waits = list(si.on_wait)
    if len(waits) <= MAX_WAITS:
        return
    si.on_wait = waits[:MAX_WAITS]
    rest = waits[MAX_WAITS:]
    while rest:
        take, rest = rest[:MAX_WAITS], rest[MAX_WAITS:]
        d2 = nc.sync.drain()
        si2 = d2.ins.sync_info
        if si2 is None:
            d2.ins.sync_info = mybir.SyncInfo(on_wait=list(take), on_update=[])
        else:
            si2.on_wait = list(si2.on_wait) + list(take)


class SplitDrainTileContext(TileContext):
    def _drain_and_barrier(self, tick_clock, wait_clock):
        from concourse.vector_clock import ScopedClock

        drain_inst = self.nc.sync.drain()
        wait_clock.add_sem_waits(
            drain_inst.ins, ScopedClock({None: tick_clock.global_clock})
        )
        _split_instruction_waits(self.nc, drain_inst)

        self.nc.all_engine_barrier()
        assert self.sems is not None
        popped = self.nc._tile_sem_poison_stack.pop()
        assert popped is self._sem_poison
        self.nc.clear_and_free_semaphores(list(self.sems.allocated().values()))
        self.nc.all_engine_barrier()
'''


def _install_tile_fix():
    if "tile_fix_embedded" in sys.modules:
        return
    mod = _types.ModuleType("tile_fix_embedded")
    exec(_tile_fix_src, mod.__dict__)
    sys.modules["tile_fix_embedded"] = mod


_install_tile_fix()


# revision 19
# speedup vs baseline: 1.2806x; 1.2806x over previous
"""Trainium2 Bass kernel for DeepDFT Message+Receiver block.

Computes, for inputs of shape
  scalar [B,A,G,F], scalar_reciever [B,P,G,F], expansion [B,P,A,E],
  mask [B,P,A,G], edge_distance [B,P,A,1], + MLP weights:

  gates = ssp(expansion @ W1e + b1e) @ W2e + b2e
  gates *= 1 - sigmoid(5*(edge_distance - (CUTOFF-1.5)))
  src = scalar @ W1n[:F]; tgt = reciever @ W1n[F:]
  nodes = ssp(src + tgt + b1n) @ W2n + b2n
  out = sum_a mask * gates * nodes          -> [B,P,G,F]

Sharding: probe axis P across 8 cores. Within a core the atom axis is
split into 4 quarters mapped onto the 4 32-partition groups (features on
partitions), columns = (b, p, a_local).

Key identities (exact):
  ssp(x) = softplus(x) - log2 = ln(exp(x - log2) + 0.5)
  exp(src+tgt+b1n-log2) = exp(src+b1n-log2) * exp(tgt)   (tiny factors)
so each ssp costs one Ln pass on ACT; the gates path needs one extra Exp
pass; all biases fold into the exponent shifts (b1e/b1n/b2e/b2n are
handled generally below).
"""
import sys, os
if "/opt/trn_rl_repo" not in sys.path:
    sys.path.insert(0, "/opt/trn_rl_repo")
os.environ.setdefault("JAX_PLATFORMS", "cpu,axon")

import numpy as np
import ml_dtypes

B, P, A, G, F, E = 2, 4096, 96, 1, 32, 20
NCORES = 8
PLOC = P // NCORES          # 512 probes per core
NGRP = 4                    # atom quarter groups
AL = A // NGRP              # 24 atoms per group
NPAIR = B * PLOC            # 1024 (b,p) pairs per core
NCOLS = NPAIR * AL          # 24576 cols per group
CH = 384                    # chunk = 16 probes * 24 atoms
CPP = CH // AL              # 16 probes per chunk
NCH = NCOLS // CH           # 64 chunks
MACC = 2                    # chunks per psum macro
NMAC = NCH // MACC          # 32 macros
SGM = 4                     # macros per tree-stage flush (4*768 = 3072 cols = 128 p)
LOG2 = 0.6931471805599453
CUTOFF = 5.0

_CACHE = {}

# Opcodes whose sem updates are executed by DMA hardware (riding the
# descriptor) rather than the issuing sequencer — their updates must not be
# moved onto a NOP.
_DMA_OPCODES = ("TensorLoad", "TensorSave", "TensorCopy", "Dge", "DMA")


def _fix_bir_json(raw: bytes) -> bytes:
    """This walrus build accepts at most ONE sem wait (and one update) per
    instruction (NEURON_ISA_TPB_EVENTS has a single wait/update slot).
    Split excess waits onto preceding same-engine NOPs (sequencer order
    makes this equivalent) and excess updates onto trailing NOPs."""
    import json
    m = json.loads(raw)
    ctr = [0]

    def mknop(engine, wait=None, upd=None):
        ctr[0] += 1
        return {
            "engine": engine, "ins": [], "outs": [],
            "name": f"I-wsplit-{ctr[0]}", "opcode": "NoOp",
            "sync_info": {
                "on_wait": [wait] if wait else [],
                "on_update": [upd] if upd else [],
            },
        }

    for fn in m["functions"]:
        for bb in fn["blocks"]:
            newl = []
            for inst in bb["instructions"]:
                si = inst.get("sync_info")
                pre, post = [], []
                if si:
                    w = si.get("on_wait") or []
                    if len(w) > 1:
                        for x in w[:-1]:
                            pre.append(mknop(inst["engine"], wait=x))
                        si["on_wait"] = [w[-1]]
                    u = si.get("on_update") or []
                    if len(u) > 1:
                        op = str(inst.get("opcode", ""))
                        assert not any(d in op for d in _DMA_OPCODES), (
                            f"multi-update DMA instruction {inst.get('name')}"
                        )
                        for x in u[1:]:
                            post.append(mknop(inst["engine"], upd=x))
                        si["on_update"] = [u[0]]
                newl.extend(pre)
                newl.append(inst)
                newl.extend(post)
            bb["instructions"] = newl
    return json.dumps(m).encode()


def _build_bass():
    import concourse.bass as bass
    import concourse.mybir as mybir
    from tile_fix_embedded import SplitDrainTileContext

    f32 = mybir.dt.float32
    bf16 = mybir.dt.bfloat16
    AF = mybir.ActivationFunctionType
    OP = mybir.AluOpType

    nc = bass.Bass(num_devices=NCORES)

    # ---- DRAM I/O ----
    d_expT = nc.dram_tensor("expT", [NGRP * E, NCOLS], bf16, kind="ExternalInput")
    d_maskq = nc.dram_tensor("maskq", [128, NCOLS // 32], f32, kind="ExternalInput")
    d_edgeq = nc.dram_tensor("edgeq", [128, NCOLS // 32], f32, kind="ExternalInput")
    d_recvT = nc.dram_tensor("recvT", [F, NPAIR], f32, kind="ExternalInput")
    d_srcT = nc.dram_tensor("srcT", [F, B * A], f32, kind="ExternalInput")
    d_bdW1e = nc.dram_tensor("bdW1e", [NGRP * E, 128], bf16, kind="ExternalInput")
    d_bdW2e = nc.dram_tensor("bdW2e", [128, 128], bf16, kind="ExternalInput")
    d_bdW2n = nc.dram_tensor("bdW2n", [128, 128], bf16, kind="ExternalInput")
    d_bdSum = nc.dram_tensor("bdSum", [128, F], f32, kind="ExternalInput")
    d_wsT = nc.dram_tensor("wsT", [F, F], f32, kind="ExternalInput")
    d_wtT = nc.dram_tensor("wtT", [F, F], f32, kind="ExternalInput")
    d_bEx = nc.dram_tensor("bEx", [128, 1], f32, kind="ExternalInput")    # b1e - log2 (x4)
    d_bEs = nc.dram_tensor("bEs", [F, 1], f32, kind="ExternalInput")      # b1n - log2
    # Ln scale/bias folds: act1 = Ln(E1*e^c + 0.5*e^c) = ssp(y1)+c, c = W2e^-T b2e
    d_lnSG = nc.dram_tensor("lnSG", [128, 1], f32, kind="ExternalInput")
    d_lnBG = nc.dram_tensor("lnBG", [128, 1], f32, kind="ExternalInput")
    d_lnSH = nc.dram_tensor("lnSH", [128, 1], f32, kind="ExternalInput")
    d_lnBH = nc.dram_tensor("lnBH", [128, 1], f32, kind="ExternalInput")
    d_out = nc.dram_tensor("outT", [F, NPAIR], f32, kind="ExternalOutput")

    with SplitDrainTileContext(nc) as tc:
        with (
            tc.tile_pool(name="persist", bufs=1) as pp,
            tc.tile_pool(name="work", bufs=3) as wp,
            tc.tile_pool(name="stage", bufs=2) as sp,
            tc.tile_pool(name="psA", bufs=2, space="PSUM") as psA,
            tc.tile_pool(name="psC", bufs=1, space="PSUM") as psC,
        ):
            # ---- persistent tiles ----
            w1e = pp.tile([NGRP * E, 128], bf16, tag="w1e")
            w2e = pp.tile([128, 128], bf16, tag="w2e")
            w2n = pp.tile([128, 128], bf16, tag="w2n")
            wsum = pp.tile([128, F], f32, tag="wsum")
            ws = pp.tile([F, F], f32, tag="ws")
            wt = pp.tile([F, F], f32, tag="wt")
            bEx = pp.tile([128, 1], f32, tag="bEx")
            bEs = pp.tile([F, 1], f32, tag="bEs")
            lnSG = pp.tile([128, 1], f32, tag="lnSG")
            lnBG = pp.tile([128, 1], f32, tag="lnBG")
            lnSH = pp.tile([128, 1], f32, tag="lnSH")
            lnBH = pp.tile([128, 1], f32, tag="lnBH")
            nc.sync.dma_start(out=w1e[:], in_=d_bdW1e[:])
            nc.sync.dma_start(out=w2e[:], in_=d_bdW2e[:])
            nc.sync.dma_start(out=w2n[:], in_=d_bdW2n[:])
            nc.sync.dma_start(out=wsum[:], in_=d_bdSum[:])
            nc.sync.dma_start(out=ws[:], in_=d_wsT[:])
            nc.sync.dma_start(out=wt[:], in_=d_wtT[:])
            nc.sync.dma_start(out=bEx[:], in_=d_bEx[:])
            nc.sync.dma_start(out=bEs[:], in_=d_bEs[:])
            nc.sync.dma_start(out=lnSG[:], in_=d_lnSG[:])
            nc.sync.dma_start(out=lnBG[:], in_=d_lnBG[:])
            nc.sync.dma_start(out=lnSH[:], in_=d_lnSH[:])
            nc.sync.dma_start(out=lnBH[:], in_=d_lnBH[:])

            # ---- s = mask * sigmoid(17.5 - 5 d): do Sigmoid FIRST (table set) ----
            mq = pp.tile([128, NCOLS // 32], f32, tag="mq")
            eq = pp.tile([128, NCOLS // 32], f32, tag="eq")
            nc.sync.dma_start(out=mq[:], in_=d_maskq[:])
            nc.sync.dma_start(out=eq[:], in_=d_edgeq[:])
            sigB = pp.tile([128, 1], f32, tag="sigB")
            nc.gpsimd.memset(sigB[:], 5.0 * (CUTOFF - 1.5))
            sig = pp.tile([128, NCOLS // 32], f32, tag="sig")
            nc.scalar.activation(sig[:], eq[:], AF.Sigmoid,
                                 bias=sigB[:, 0:1], scale=-5.0)
            sqb = pp.tile([128, NCOLS // 32], bf16, tag="sqb")
            nc.vector.tensor_mul(out=sqb[:], in0=mq[:], in1=sig[:])

            # ---- S_all [128, NCOLS] bf16: row (32i+h) holds group i's s-vector ----
            # Bounce sqb through DRAM to linearize each group's 32 rows into
            # one row, then log-double across partitions (5 DMAs per group).
            d_sbounce = nc.dram_tensor("sbounce", [NGRP, NCOLS], bf16)
            S_all = pp.tile([128, NCOLS], bf16, tag="S_all")
            nc.gpsimd.dma_start(
                out=d_sbounce[:].rearrange("i (k m) -> (i k) m", k=32),
                in_=sqb[:])
            for i in range(NGRP):
                nc.gpsimd.dma_start(
                    out=S_all[32 * i : 32 * i + 1, :],
                    in_=d_sbounce[i : i + 1, :])
                rep = 1
                while rep < 32:
                    nc.gpsimd.dma_start(
                        out=S_all[32 * i + rep : 32 * i + 2 * rep, :],
                        in_=S_all[32 * i : 32 * i + rep, :],
                    )
                    rep *= 2

            # ---- es4 [128, B*AL], et4 [128, NPAIR] (bf16, exp domain) ----
            srcT = pp.tile([F, B * A], f32, tag="srcT")
            recvT = pp.tile([F, NPAIR], f32, tag="recvT")
            nc.sync.dma_start(out=srcT[:], in_=d_srcT[:])
            nc.sync.dma_start(out=recvT[:], in_=d_recvT[:])

            ps_s = psC.tile([F, B * A], f32, tag="psG")
            nc.tensor.matmul(ps_s[:], ws[:], srcT[:], start=True, stop=True)
            es_full = pp.tile([F, B * A], bf16, tag="es_full")
            nc.scalar.activation(es_full[:], ps_s[:], AF.Exp, bias=bEs[:, 0:1])

            et_full = pp.tile([F, NPAIR], bf16, tag="et_full")
            for half in range(2):
                ps_t = psC.tile([F, 512], f32, tag="psN")
                nc.tensor.matmul(ps_t[:], wt[:], recvT[:, 512 * half : 512 * (half + 1)],
                                 start=True, stop=True)
                nc.scalar.activation(et_full[:, 512 * half : 512 * (half + 1)],
                                     ps_t[:], AF.Exp)

            es4 = pp.tile([128, B * AL], bf16, tag="es4")
            et4 = pp.tile([128, NPAIR], bf16, tag="et4")
            for i in range(NGRP):
                for b in range(B):
                    nc.sync.dma_start(
                        out=es4[32 * i : 32 * i + 32, b * AL : (b + 1) * AL],
                        in_=es_full[:, b * A + AL * i : b * A + AL * (i + 1)],
                    )
                nc.sync.dma_start(out=et4[32 * i : 32 * i + 32, :], in_=et_full[:])

            # ---- output accumulator ----
            OUT4 = pp.tile([128, NPAIR], f32, tag="OUT4")

            # ---- main loop ----
            MW = MACC * CH  # 768 macro width
            for sg in range(NMAC // SGM):  # stage groups of SGM macros
                stage = sp.tile([128, SGM * MW], bf16, tag="stage")
                for mi in range(SGM):
                    m = sg * SGM + mi
                    bidx = (m * MW) // (PLOC * AL)          # which b
                    poff = ((m * MW) % (PLOC * AL)) // AL   # probe offset in b
                    npch = MW // AL                          # 32 probes per macro

                    X = wp.tile([NGRP * E, MW], bf16, tag="X")
                    nc.sync.dma_start(out=X[:], in_=d_expT[:, m * MW : (m + 1) * MW])

                    ps1 = psA.tile([128, 1024], f32, tag="ps1")
                    for c in range(MACC):
                        nc.tensor.matmul(
                            ps1[:, 512 * c : 512 * c + CH],
                            w1e[:], X[:, CH * c : CH * (c + 1)],
                            start=True, stop=True)
                    ps1v = ps1[:].rearrange("p (c w) -> p c w", c=MACC)[:, :, 0:CH]
                    E1 = wp.tile([128, MW], f32, tag="E1")
                    E1v = E1[:].rearrange("p (c w) -> p c w", c=MACC)
                    nc.scalar.activation(E1v, ps1v, AF.Exp, bias=bEx[:, 0:1])

                    act1 = wp.tile([128, MW], bf16, tag="act1")
                    nc.scalar.activation(act1[:], E1[:], AF.Ln,
                                         bias=lnBG[:, 0:1], scale=lnSG[:, 0:1])

                    psG = psC.tile([128, 1024], f32, tag="psG")
                    for c in range(MACC):
                        nc.tensor.matmul(
                            psG[:, 512 * c : 512 * c + CH],
                            w2e[:], act1[:, CH * c : CH * (c + 1)],
                            start=True, stop=True)

                    # ehp = es4 * et4 (broadcast views), bf16
                    ehp = wp.tile([128, MW], bf16, tag="ehp")
                    ehpv = ehp[:].rearrange("p (q w) -> p q w", q=npch)
                    esv = es4[:, None, bidx * AL : (bidx + 1) * AL].broadcast_to(
                        [128, npch, AL])
                    etv = et4[:, bidx * PLOC + poff : bidx * PLOC + poff + npch, None
                              ].broadcast_to([128, npch, AL])
                    nc.vector.tensor_mul(out=ehpv, in0=esv, in1=etv)

                    actH = wp.tile([128, MW], bf16, tag="actH")
                    nc.scalar.activation(actH[:], ehp[:], AF.Ln,
                                         bias=lnBH[:, 0:1], scale=lnSH[:, 0:1])

                    psN = psC.tile([128, 1024], f32, tag="psN")
                    for c in range(MACC):
                        nc.tensor.matmul(
                            psN[:, 512 * c : 512 * c + CH],
                            w2n[:], actH[:, CH * c : CH * (c + 1)],
                            start=True, stop=True)

                    # sq = (G * s) * N  — DVE can read only one PSUM input
                    # per op, so s (SBUF) pairs with G, then N.
                    psGv = psG[:].rearrange("p (c w) -> p c w", c=MACC)[:, :, 0:CH]
                    psNv = psN[:].rearrange("p (c w) -> p c w", c=MACC)[:, :, 0:CH]
                    Sv = S_all[:, m * MW : (m + 1) * MW].rearrange(
                        "p (c w) -> p c w", c=MACC)
                    gs = wp.tile([128, MW], bf16, tag="q")
                    gsv = gs[:].rearrange("p (c w) -> p c w", c=MACC)
                    nc.vector.tensor_mul(out=gsv, in0=psGv, in1=Sv)
                    sqv = stage[:, mi * MW : (mi + 1) * MW].rearrange(
                        "p (c w) -> p c w", c=MACC)
                    nc.vector.tensor_mul(out=sqv, in0=gsv, in1=psNv)

                # tree-reduce stage [128, SGM*MW] over a_local (24)
                NPS = SGM * MW // AL  # 128 probes
                sv = stage[:].rearrange("p (n a) -> p n a", a=AL)
                t1 = sp.tile([128, NPS * 12], bf16, tag="t1")
                t1v = t1[:].rearrange("p (n a) -> p n a", a=12)
                nc.vector.tensor_add(out=t1v, in0=sv[:, :, 0:12], in1=sv[:, :, 12:24])
                t2 = sp.tile([128, NPS * 6], bf16, tag="t2")
                t2v = t2[:].rearrange("p (n a) -> p n a", a=6)
                nc.vector.tensor_add(out=t2v, in0=t1v[:, :, 0:6], in1=t1v[:, :, 6:12])
                t3 = sp.tile([128, NPS * 3], bf16, tag="t3")
                t3v = t3[:].rearrange("p (n a) -> p n a", a=3)
                nc.vector.tensor_add(out=t3v, in0=t2v[:, :, 0:3], in1=t2v[:, :, 3:6])
                t4 = sp.tile([128, NPS], f32, tag="t4")
                t4v = t4[:].rearrange("p (n a) -> p n a", a=1)
                nc.vector.tensor_add(out=t4v, in0=t3v[:, :, 0:1], in1=t3v[:, :, 1:2])
                pbase = sg * NPS
                ov = OUT4[:, pbase : pbase + NPS].rearrange("p (n a) -> p n a", a=1)
                nc.vector.tensor_add(out=ov, in0=t4v, in1=t3v[:, :, 2:3])

            # ---- cross-group sum + writeout ----
            outsb = pp.tile([F, NPAIR], f32, tag="outsb")
            for half in range(2):
                psF = psC.tile([F, 512], f32, tag="psN")
                nc.tensor.matmul(psF[:], wsum[:],
                                 OUT4[:, 512 * half : 512 * (half + 1)],
                                 start=True, stop=True)
                nc.vector.tensor_copy(outsb[:, 512 * half : 512 * (half + 1)], psF[:])
            nc.sync.dma_start(out=d_out[:], in_=outsb[:])

    # Patch serialization: enforce the 1-wait/1-update ISA slot limit.
    import types
    _orig_tjb = nc.to_json_bytes
    _fixed = {}

    def _patched_to_json_bytes(self):
        if "b" not in _fixed:
            _fixed["b"] = _fix_bir_json(_orig_tjb())
        return _fixed["b"]

    nc.to_json_bytes = types.MethodType(_patched_to_json_bytes, nc)
    return nc


def _host_prep(inputs):
    """Host-side layout prep: slicing/transpose/padding only (plus constant
    folds on the tiny weight matrices)."""
    scalar = np.asarray(inputs["scalar"], np.float32)
    reciever = np.asarray(inputs["scalar_reciever"], np.float32)
    expansion = np.asarray(inputs["expansion"], np.float32)
    mask = np.asarray(inputs["mask"], np.float32)
    edge = np.asarray(inputs["edge_distance"], np.float32)
    W1e = np.asarray(inputs["W1e"], np.float32)
    b1e = np.asarray(inputs["b1e"], np.float32)
    W2e = np.asarray(inputs["W2e"], np.float32)
    b2e = np.asarray(inputs["b2e"], np.float32)
    W1n = np.asarray(inputs["W1n"], np.float32)
    b1n = np.asarray(inputs["b1n"], np.float32)
    W2n = np.asarray(inputs["W2n"], np.float32)
    b2n = np.asarray(inputs["b2n"], np.float32)

    bdW1e = np.zeros((NGRP * E, 128), np.float32)
    bdW2e = np.zeros((128, 128), np.float32)
    bdW2n = np.zeros((128, 128), np.float32)
    bdSum = np.zeros((128, F), np.float32)
    for i in range(NGRP):
        bdW1e[i * E : (i + 1) * E, 32 * i : 32 * i + F] = W1e
        bdW2e[32 * i : 32 * i + F, 32 * i : 32 * i + F] = W2e
        bdW2n[32 * i : 32 * i + F, 32 * i : 32 * i + F] = W2n
        bdSum[32 * i : 32 * i + F, :] = np.eye(F, dtype=np.float32)
    # act1 = Ln(E1*e^cg + 0.5*e^cg) = ssp(y1) + cg with cg = W2e^-T b2e, so
    # act1 @ W2e = ssp @ W2e + b2e exactly (same for the nodes path).
    cg = np.linalg.solve(W2e.T.astype(np.float64), b2e.astype(np.float64))
    cn = np.linalg.solve(W2n.T.astype(np.float64), b2n.astype(np.float64))
    bf = ml_dtypes.bfloat16
    shared = {
        "bdW1e": bdW1e.astype(bf), "bdW2e": bdW2e.astype(bf),
        "bdW2n": bdW2n.astype(bf), "bdSum": bdSum,
        "wsT": np.ascontiguousarray(W1n[:F]),
        "wtT": np.ascontiguousarray(W1n[F:]),
        "bEx": np.ascontiguousarray((np.tile(b1e, NGRP) - LOG2)[:, None]),
        "bEs": np.ascontiguousarray((b1n - LOG2)[:, None]),
        "lnSG": np.tile(np.exp(cg), NGRP).astype(np.float32)[:, None].copy(),
        "lnBG": np.tile(0.5 * np.exp(cg), NGRP).astype(np.float32)[:, None].copy(),
        "lnSH": np.tile(np.exp(cn), NGRP).astype(np.float32)[:, None].copy(),
        "lnBH": np.tile(0.5 * np.exp(cn), NGRP).astype(np.float32)[:, None].copy(),
    }
    srcT = np.ascontiguousarray(scalar[:, :, 0, :].reshape(B * A, F).T)

    in_maps = []
    for c in range(NCORES):
        psl = slice(c * PLOC, (c + 1) * PLOC)
        x = expansion[:, psl].reshape(B, PLOC, NGRP, AL, E)
        expT = np.ascontiguousarray(
            x.transpose(2, 4, 0, 1, 3).reshape(NGRP * E, NCOLS)).astype(
                ml_dtypes.bfloat16)
        mq = np.ascontiguousarray(
            mask[:, psl, :, 0].reshape(B, PLOC, NGRP, AL)
            .transpose(2, 0, 1, 3).reshape(128, NCOLS // 32))
        eq = np.ascontiguousarray(
            edge[:, psl, :, 0].reshape(B, PLOC, NGRP, AL)
            .transpose(2, 0, 1, 3).reshape(128, NCOLS // 32))
        recvT = np.ascontiguousarray(
            reciever[:, psl, 0, :].reshape(NPAIR, F).T)
        in_maps.append({
            "expT": expT, "maskq": mq, "edgeq": eq,
            "recvT": recvT, "srcT": srcT, **shared,
        })
    return in_maps


def kernel(**inputs):
    if "nc" not in _CACHE:
        _CACHE["nc"] = _build_bass()
    nc = _CACHE["nc"]
    in_maps = _host_prep(inputs)

    from concourse.bass_utils import run_bass_kernel_spmd
    trace = os.environ.get("BASS_KERNEL_TRACE", "0") == "1"
    res = run_bass_kernel_spmd(nc, in_maps, core_ids=list(range(NCORES)),
                               trace=trace)
    _CACHE["last_result"] = res

    out = np.empty((B, P, G, F), np.float32)
    for c in range(NCORES):
        outT = res.results[c]["outT"]            # [F, NPAIR]
        out[:, c * PLOC : (c + 1) * PLOC, 0, :] = outT.T.reshape(B, PLOC, F)
    return out


# --- embedded TileContext fix (kernel.py must be self-contained) ---
import types as _types

_tile_fix_src = '''
import concourse.mybir as mybir
from concourse.tile import TileContext

MAX_WAITS = 1


def _split_instruction_waits(nc, drain_inst):
    si = drain_inst.ins.sync_info
    if si is None:
        return
    waits = list(si.on_wait)
    if len(waits) <= MAX_WAITS:
        return
    si.on_wait = waits[:MAX_WAITS]
    rest = waits[MAX_WAITS:]
    while rest:
        take, rest = rest[:MAX_WAITS], rest[MAX_WAITS:]
        d2 = nc.sync.drain()
        si2 = d2.ins.sync_info
        if si2 is None:
            d2.ins.sync_info = mybir.SyncInfo(on_wait=list(take), on_update=[])
        else:
            si2.on_wait = list(si2.on_wait) + list(take)


class SplitDrainTileContext(TileContext):
    def _drain_and_barrier(self, tick_clock, wait_clock):
        from concourse.vector_clock import ScopedClock

        drain_inst = self.nc.sync.drain()
        wait_clock.add_sem_waits(
            drain_inst.ins, ScopedClock({None: tick_clock.global_clock})
        )
        _split_instruction_waits(self.nc, drain_inst)

        self.nc.all_engine_barrier()
        assert self.sems is not None
        popped = self.nc._tile_sem_poison_stack.pop()
        assert popped is self._sem_poison
        self.nc.clear_and_free_semaphores(list(self.sems.allocated().values()))
        self.nc.all_engine_barrier()
'''


def _install_tile_fix():
    if "tile_fix_embedded" in sys.modules:
        return
    mod = _types.ModuleType("tile_fix_embedded")
    exec(_tile_fix_src, mod.__dict__)
    sys.modules["tile_fix_embedded"] = mod


_install_tile_fix()


# revision 22
# speedup vs baseline: 1.3050x; 1.0191x over previous
"""Trainium2 Bass kernel for DeepDFT Message+Receiver block.

Computes, for inputs of shape
  scalar [B,A,G,F], scalar_reciever [B,P,G,F], expansion [B,P,A,E],
  mask [B,P,A,G], edge_distance [B,P,A,1], + MLP weights:

  gates = ssp(expansion @ W1e + b1e) @ W2e + b2e
  gates *= 1 - sigmoid(5*(edge_distance - (CUTOFF-1.5)))
  src = scalar @ W1n[:F]; tgt = reciever @ W1n[F:]
  nodes = ssp(src + tgt + b1n) @ W2n + b2n
  out = sum_a mask * gates * nodes          -> [B,P,G,F]

Sharding: probe axis P across 8 cores. Within a core the atom axis is
split into 4 quarters mapped onto the 4 32-partition groups (features on
partitions), columns = (b, p, a_local).

Key identities (exact):
  ssp(x) = softplus(x) - log2 = ln(exp(x - log2) + 0.5)
  exp(src+tgt+b1n-log2) = exp(src+b1n-log2) * exp(tgt)   (tiny factors)
so each ssp costs one Ln pass on ACT; the gates path needs one extra Exp
pass; all biases fold into the exponent shifts (b1e/b1n/b2e/b2n are
handled generally below).
"""
import sys, os
if "/opt/trn_rl_repo" not in sys.path:
    sys.path.insert(0, "/opt/trn_rl_repo")
os.environ.setdefault("JAX_PLATFORMS", "cpu,axon")

import numpy as np
import ml_dtypes

B, P, A, G, F, E = 2, 4096, 96, 1, 32, 20
NCORES = 8
PLOC = P // NCORES          # 512 probes per core
NGRP = 4                    # atom quarter groups
AL = A // NGRP              # 24 atoms per group
NPAIR = B * PLOC            # 1024 (b,p) pairs per core
NCOLS = NPAIR * AL          # 24576 cols per group
CH = 384                    # chunk = 16 probes * 24 atoms
CPP = CH // AL              # 16 probes per chunk
NCH = NCOLS // CH           # 64 chunks
MACC = 4                    # chunks per psum macro
NMAC = NCH // MACC          # 32 macros
SGM = 2                     # macros per tree-stage flush (3072 cols = 128 p)
LOG2 = 0.6931471805599453
CUTOFF = 5.0

_CACHE = {}

# Opcodes whose sem updates are executed by DMA hardware (riding the
# descriptor) rather than the issuing sequencer — their updates must not be
# moved onto a NOP.
_DMA_OPCODES = ("TensorLoad", "TensorSave", "TensorCopy", "Dge", "DMA")


def _fix_bir_json(raw: bytes) -> bytes:
    """This walrus build accepts at most ONE sem wait (and one update) per
    instruction (NEURON_ISA_TPB_EVENTS has a single wait/update slot).
    Split excess waits onto preceding same-engine NOPs (sequencer order
    makes this equivalent) and excess updates onto trailing NOPs."""
    import json
    m = json.loads(raw)
    ctr = [0]

    def mknop(engine, wait=None, upd=None):
        ctr[0] += 1
        return {
            "engine": engine, "ins": [], "outs": [],
            "name": f"I-wsplit-{ctr[0]}", "opcode": "NoOp",
            "sync_info": {
                "on_wait": [wait] if wait else [],
                "on_update": [upd] if upd else [],
            },
        }

    for fn in m["functions"]:
        for bb in fn["blocks"]:
            newl = []
            for inst in bb["instructions"]:
                si = inst.get("sync_info")
                pre, post = [], []
                if si:
                    w = si.get("on_wait") or []
                    if len(w) > 1:
                        for x in w[:-1]:
                            pre.append(mknop(inst["engine"], wait=x))
                        si["on_wait"] = [w[-1]]
                    u = si.get("on_update") or []
                    if len(u) > 1:
                        op = str(inst.get("opcode", ""))
                        assert not any(d in op for d in _DMA_OPCODES), (
                            f"multi-update DMA instruction {inst.get('name')}"
                        )
                        for x in u[1:]:
                            post.append(mknop(inst["engine"], upd=x))
                        si["on_update"] = [u[0]]
                newl.extend(pre)
                newl.append(inst)
                newl.extend(post)
            bb["instructions"] = newl
    return json.dumps(m).encode()


def _build_bass():
    import concourse.bass as bass
    import concourse.mybir as mybir
    from tile_fix_embedded import SplitDrainTileContext

    f32 = mybir.dt.float32
    bf16 = mybir.dt.bfloat16
    AF = mybir.ActivationFunctionType
    OP = mybir.AluOpType

    nc = bass.Bass(num_devices=NCORES)

    # ---- DRAM I/O ----
    d_expT = nc.dram_tensor("expT", [NGRP * E, NCOLS], bf16, kind="ExternalInput")
    d_maskq = nc.dram_tensor("maskq", [128, NCOLS // 32], f32, kind="ExternalInput")
    d_edgeq = nc.dram_tensor("edgeq", [128, NCOLS // 32], f32, kind="ExternalInput")
    d_recvT = nc.dram_tensor("recvT", [F, NPAIR], f32, kind="ExternalInput")
    d_srcT = nc.dram_tensor("srcT", [F, B * A], f32, kind="ExternalInput")
    d_bdW1e = nc.dram_tensor("bdW1e", [NGRP * E, 128], bf16, kind="ExternalInput")
    d_bdW2e = nc.dram_tensor("bdW2e", [128, 128], bf16, kind="ExternalInput")
    d_bdW2n = nc.dram_tensor("bdW2n", [128, 128], bf16, kind="ExternalInput")
    d_bdSum = nc.dram_tensor("bdSum", [128, F], f32, kind="ExternalInput")
    d_wsT = nc.dram_tensor("wsT", [F, F], f32, kind="ExternalInput")
    d_wtT = nc.dram_tensor("wtT", [F, F], f32, kind="ExternalInput")
    d_bEx = nc.dram_tensor("bEx", [128, 1], f32, kind="ExternalInput")    # b1e - log2 (x4)
    d_bEs = nc.dram_tensor("bEs", [F, 1], f32, kind="ExternalInput")      # b1n - log2
    # Ln scale/bias folds: act1 = Ln(E1*e^c + 0.5*e^c) = ssp(y1)+c, c = W2e^-T b2e
    d_lnSG = nc.dram_tensor("lnSG", [128, 1], f32, kind="ExternalInput")
    d_lnBG = nc.dram_tensor("lnBG", [128, 1], f32, kind="ExternalInput")
    d_lnSH = nc.dram_tensor("lnSH", [128, 1], f32, kind="ExternalInput")
    d_lnBH = nc.dram_tensor("lnBH", [128, 1], f32, kind="ExternalInput")
    d_out = nc.dram_tensor("outT", [F, NPAIR], f32, kind="ExternalOutput")

    with SplitDrainTileContext(nc) as tc:
        with (
            tc.tile_pool(name="persist", bufs=1) as pp,
            tc.tile_pool(name="work", bufs=3) as wp,
            tc.tile_pool(name="stage", bufs=2) as sp,
            tc.tile_pool(name="psA", bufs=1, space="PSUM") as psA,
            tc.tile_pool(name="psC", bufs=2, space="PSUM") as psC,
        ):
            # ---- persistent tiles ----
            w1e = pp.tile([NGRP * E, 128], bf16, tag="w1e")
            w2e = pp.tile([128, 128], bf16, tag="w2e")
            w2n = pp.tile([128, 128], bf16, tag="w2n")
            wsum = pp.tile([128, F], f32, tag="wsum")
            ws = pp.tile([F, F], f32, tag="ws")
            wt = pp.tile([F, F], f32, tag="wt")
            bEx = pp.tile([128, 1], f32, tag="bEx")
            bEs = pp.tile([F, 1], f32, tag="bEs")
            lnSG = pp.tile([128, 1], f32, tag="lnSG")
            lnBG = pp.tile([128, 1], f32, tag="lnBG")
            lnSH = pp.tile([128, 1], f32, tag="lnSH")
            lnBH = pp.tile([128, 1], f32, tag="lnBH")
            nc.sync.dma_start(out=w1e[:], in_=d_bdW1e[:])
            nc.sync.dma_start(out=w2e[:], in_=d_bdW2e[:])
            nc.sync.dma_start(out=w2n[:], in_=d_bdW2n[:])
            nc.sync.dma_start(out=wsum[:], in_=d_bdSum[:])
            nc.sync.dma_start(out=ws[:], in_=d_wsT[:])
            nc.sync.dma_start(out=wt[:], in_=d_wtT[:])
            nc.sync.dma_start(out=bEx[:], in_=d_bEx[:])
            nc.sync.dma_start(out=bEs[:], in_=d_bEs[:])
            nc.sync.dma_start(out=lnSG[:], in_=d_lnSG[:])
            nc.sync.dma_start(out=lnBG[:], in_=d_lnBG[:])
            nc.sync.dma_start(out=lnSH[:], in_=d_lnSH[:])
            nc.sync.dma_start(out=lnBH[:], in_=d_lnBH[:])

            # ---- s = mask * sigmoid(17.5 - 5 d): do Sigmoid FIRST (table set) ----
            mq = pp.tile([128, NCOLS // 32], f32, tag="mq")
            eq = pp.tile([128, NCOLS // 32], f32, tag="eq")
            nc.sync.dma_start(out=mq[:], in_=d_maskq[:])
            nc.sync.dma_start(out=eq[:], in_=d_edgeq[:])
            sigB = pp.tile([128, 1], f32, tag="sigB")
            nc.gpsimd.memset(sigB[:], 5.0 * (CUTOFF - 1.5))
            sig = pp.tile([128, NCOLS // 32], f32, tag="sig")
            nc.scalar.activation(sig[:], eq[:], AF.Sigmoid,
                                 bias=sigB[:, 0:1], scale=-5.0)
            sqb = pp.tile([128, NCOLS // 32], bf16, tag="sqb")
            nc.vector.tensor_mul(out=sqb[:], in0=mq[:], in1=sig[:])

            # ---- S_all [128, NCOLS] bf16: row (32i+h) holds group i's s-vector ----
            # Bounce sqb through DRAM to linearize each group's 32 rows into
            # one row, then log-double across partitions (5 DMAs per group).
            d_sbounce = nc.dram_tensor("sbounce", [NGRP, NCOLS], bf16)
            S_all = pp.tile([128, NCOLS], bf16, tag="S_all")
            nc.gpsimd.dma_start(
                out=d_sbounce[:].rearrange("i (k m) -> (i k) m", k=32),
                in_=sqb[:])
            for i in range(NGRP):
                nc.gpsimd.dma_start(
                    out=S_all[32 * i : 32 * i + 1, :],
                    in_=d_sbounce[i : i + 1, :])
                rep = 1
                while rep < 32:
                    nc.gpsimd.dma_start(
                        out=S_all[32 * i + rep : 32 * i + 2 * rep, :],
                        in_=S_all[32 * i : 32 * i + rep, :],
                    )
                    rep *= 2

            # ---- es4 [128, B*AL], et4 [128, NPAIR] (bf16, exp domain) ----
            srcT = pp.tile([F, B * A], f32, tag="srcT")
            recvT = pp.tile([F, NPAIR], f32, tag="recvT")
            nc.sync.dma_start(out=srcT[:], in_=d_srcT[:])
            nc.sync.dma_start(out=recvT[:], in_=d_recvT[:])

            ps_s = psC.tile([F, B * A], f32, tag="psG")
            nc.tensor.matmul(ps_s[:], ws[:], srcT[:], start=True, stop=True)
            es_full = pp.tile([F, B * A], bf16, tag="es_full")
            nc.scalar.activation(es_full[:], ps_s[:], AF.Exp, bias=bEs[:, 0:1])

            et_full = pp.tile([F, NPAIR], bf16, tag="et_full")
            for half in range(2):
                ps_t = psC.tile([F, 512], f32, tag="psN")
                nc.tensor.matmul(ps_t[:], wt[:], recvT[:, 512 * half : 512 * (half + 1)],
                                 start=True, stop=True)
                nc.scalar.activation(et_full[:, 512 * half : 512 * (half + 1)],
                                     ps_t[:], AF.Exp)

            es4 = pp.tile([128, B * AL], bf16, tag="es4")
            et4 = pp.tile([128, NPAIR], bf16, tag="et4")
            for i in range(NGRP):
                for b in range(B):
                    nc.sync.dma_start(
                        out=es4[32 * i : 32 * i + 32, b * AL : (b + 1) * AL],
                        in_=es_full[:, b * A + AL * i : b * A + AL * (i + 1)],
                    )
                nc.sync.dma_start(out=et4[32 * i : 32 * i + 32, :], in_=et_full[:])

            # ---- output accumulator ----
            OUT4 = pp.tile([128, NPAIR], f32, tag="OUT4")

            # ---- main loop ----
            MW = MACC * CH  # macro width
            for sg in range(NMAC // SGM):  # stage groups of SGM macros
                stage = sp.tile([128, SGM * MW], bf16, tag="stage")
                for mi in range(SGM):
                    m = sg * SGM + mi
                    bidx = (m * MW) // (PLOC * AL)          # which b
                    poff = ((m * MW) % (PLOC * AL)) // AL   # probe offset in b
                    npch = MW // AL                          # probes per macro

                    X = wp.tile([NGRP * E, MW], bf16, tag="X")
                    nc.sync.dma_start(out=X[:], in_=d_expT[:, m * MW : (m + 1) * MW])

                    ps1 = psA.tile([128, 512 * MACC], f32, tag="ps1")
                    for c in range(MACC):
                        nc.tensor.matmul(
                            ps1[:, 512 * c : 512 * c + CH],
                            w1e[:], X[:, CH * c : CH * (c + 1)],
                            start=True, stop=True)
                    ps1v = ps1[:].rearrange("p (c w) -> p c w", c=MACC)[:, :, 0:CH]
                    E1 = wp.tile([128, MW], f32, tag="E1")
                    E1v = E1[:].rearrange("p (c w) -> p c w", c=MACC)
                    nc.scalar.activation(E1v, ps1v, AF.Exp, bias=bEx[:, 0:1])

                    act1 = wp.tile([128, MW], bf16, tag="act1")
                    nc.scalar.activation(act1[:], E1[:], AF.Ln,
                                         bias=lnBG[:, 0:1], scale=lnSG[:, 0:1])

                    # ehp = es4 * et4 (broadcast views), bf16
                    ehp = wp.tile([128, MW], bf16, tag="ehp")
                    ehpv = ehp[:].rearrange("p (q w) -> p q w", q=npch)
                    esv = es4[:, None, bidx * AL : (bidx + 1) * AL].broadcast_to(
                        [128, npch, AL])
                    etv = et4[:, bidx * PLOC + poff : bidx * PLOC + poff + npch, None
                              ].broadcast_to([128, npch, AL])
                    nc.vector.tensor_mul(out=ehpv, in0=esv, in1=etv)

                    actH = wp.tile([128, MW], bf16, tag="actH")
                    nc.scalar.activation(actH[:], ehp[:], AF.Ln,
                                         bias=lnBH[:, 0:1], scale=lnSH[:, 0:1])

                    # per-chunk mm2/mmH with 1-bank double-buffered psum so
                    # PE and DVE ping-pong without serializing
                    for c in range(MACC):
                        psG = psC.tile([128, 512], f32, tag="psG")
                        nc.tensor.matmul(
                            psG[:, 0:CH],
                            w2e[:], act1[:, CH * c : CH * (c + 1)],
                            start=True, stop=True)
                        psN = psC.tile([128, 512], f32, tag="psN")
                        nc.tensor.matmul(
                            psN[:, 0:CH],
                            w2n[:], actH[:, CH * c : CH * (c + 1)],
                            start=True, stop=True)
                        # sq = (G * s) * N — DVE reads one PSUM input per op
                        cw = m * MW + c * CH
                        gs = wp.tile([128, CH], bf16, tag="q")
                        nc.vector.tensor_mul(
                            out=gs[:], in0=psG[:, 0:CH],
                            in1=S_all[:, cw : cw + CH])
                        nc.vector.tensor_mul(
                            out=stage[:, mi * MW + c * CH : mi * MW + (c + 1) * CH],
                            in0=gs[:], in1=psN[:, 0:CH])

                # tree-reduce stage [128, SGM*MW] over a_local (24)
                NPS = SGM * MW // AL  # 128 probes
                sv = stage[:].rearrange("p (n a) -> p n a", a=AL)
                t1 = sp.tile([128, NPS * 12], bf16, tag="t1")
                t1v = t1[:].rearrange("p (n a) -> p n a", a=12)
                nc.vector.tensor_add(out=t1v, in0=sv[:, :, 0:12], in1=sv[:, :, 12:24])
                t2 = sp.tile([128, NPS * 6], bf16, tag="t2")
                t2v = t2[:].rearrange("p (n a) -> p n a", a=6)
                nc.vector.tensor_add(out=t2v, in0=t1v[:, :, 0:6], in1=t1v[:, :, 6:12])
                t3 = sp.tile([128, NPS * 3], bf16, tag="t3")
                t3v = t3[:].rearrange("p (n a) -> p n a", a=3)
                nc.vector.tensor_add(out=t3v, in0=t2v[:, :, 0:3], in1=t2v[:, :, 3:6])
                t4 = sp.tile([128, NPS], f32, tag="t4")
                t4v = t4[:].rearrange("p (n a) -> p n a", a=1)
                nc.vector.tensor_add(out=t4v, in0=t3v[:, :, 0:1], in1=t3v[:, :, 1:2])
                pbase = sg * NPS
                ov = OUT4[:, pbase : pbase + NPS].rearrange("p (n a) -> p n a", a=1)
                nc.vector.tensor_add(out=ov, in0=t4v, in1=t3v[:, :, 2:3])

            # ---- cross-group sum + writeout ----
            outsb = pp.tile([F, NPAIR], f32, tag="outsb")
            for half in range(2):
                psF = psC.tile([F, 512], f32, tag="psN")
                nc.tensor.matmul(psF[:], wsum[:],
                                 OUT4[:, 512 * half : 512 * (half + 1)],
                                 start=True, stop=True)
                nc.vector.tensor_copy(outsb[:, 512 * half : 512 * (half + 1)], psF[:])
            nc.sync.dma_start(out=d_out[:], in_=outsb[:])

    # Patch serialization: enforce the 1-wait/1-update ISA slot limit.
    import types
    _orig_tjb = nc.to_json_bytes
    _fixed = {}

    def _patched_to_json_bytes(self):
        if "b" not in _fixed:
            _fixed["b"] = _fix_bir_json(_orig_tjb())
        return _fixed["b"]

    nc.to_json_bytes = types.MethodType(_patched_to_json_bytes, nc)
    return nc


def _host_prep(inputs):
    """Host-side layout prep: slicing/transpose/padding only (plus constant
    folds on the tiny weight matrices)."""
    scalar = np.asarray(inputs["scalar"], np.float32)
    reciever = np.asarray(inputs["scalar_reciever"], np.float32)
    expansion = np.asarray(inputs["expansion"], np.float32)
    mask = np.asarray(inputs["mask"], np.float32)
    edge = np.asarray(inputs["edge_distance"], np.float32)
    W1e = np.asarray(inputs["W1e"], np.float32)
    b1e = np.asarray(inputs["b1e"], np.float32)
    W2e = np.asarray(inputs["W2e"], np.float32)
    b2e = np.asarray(inputs["b2e"], np.float32)
    W1n = np.asarray(inputs["W1n"], np.float32)
    b1n = np.asarray(inputs["b1n"], np.float32)
    W2n = np.asarray(inputs["W2n"], np.float32)
    b2n = np.asarray(inputs["b2n"], np.float32)

    bdW1e = np.zeros((NGRP * E, 128), np.float32)
    bdW2e = np.zeros((128, 128), np.float32)
    bdW2n = np.zeros((128, 128), np.float32)
    bdSum = np.zeros((128, F), np.float32)
    for i in range(NGRP):
        bdW1e[i * E : (i + 1) * E, 32 * i : 32 * i + F] = W1e
        bdW2e[32 * i : 32 * i + F, 32 * i : 32 * i + F] = W2e
        bdW2n[32 * i : 32 * i + F, 32 * i : 32 * i + F] = W2n
        bdSum[32 * i : 32 * i + F, :] = np.eye(F, dtype=np.float32)
    # act1 = Ln(E1*e^cg + 0.5*e^cg) = ssp(y1) + cg with cg = W2e^-T b2e, so
    # act1 @ W2e = ssp @ W2e + b2e exactly (same for the nodes path).
    cg = np.linalg.solve(W2e.T.astype(np.float64), b2e.astype(np.float64))
    cn = np.linalg.solve(W2n.T.astype(np.float64), b2n.astype(np.float64))
    bf = ml_dtypes.bfloat16
    shared = {
        "bdW1e": bdW1e.astype(bf), "bdW2e": bdW2e.astype(bf),
        "bdW2n": bdW2n.astype(bf), "bdSum": bdSum,
        "wsT": np.ascontiguousarray(W1n[:F]),
        "wtT": np.ascontiguousarray(W1n[F:]),
        "bEx": np.ascontiguousarray((np.tile(b1e, NGRP) - LOG2)[:, None]),
        "bEs": np.ascontiguousarray((b1n - LOG2)[:, None]),
        "lnSG": np.tile(np.exp(cg), NGRP).astype(np.float32)[:, None].copy(),
        "lnBG": np.tile(0.5 * np.exp(cg), NGRP).astype(np.float32)[:, None].copy(),
        "lnSH": np.tile(np.exp(cn), NGRP).astype(np.float32)[:, None].copy(),
        "lnBH": np.tile(0.5 * np.exp(cn), NGRP).astype(np.float32)[:, None].copy(),
    }
    srcT = np.ascontiguousarray(scalar[:, :, 0, :].reshape(B * A, F).T)

    in_maps = []
    for c in range(NCORES):
        psl = slice(c * PLOC, (c + 1) * PLOC)
        x = expansion[:, psl].reshape(B, PLOC, NGRP, AL, E)
        expT = np.ascontiguousarray(
            x.transpose(2, 4, 0, 1, 3).reshape(NGRP * E, NCOLS)).astype(
                ml_dtypes.bfloat16)
        mq = np.ascontiguousarray(
            mask[:, psl, :, 0].reshape(B, PLOC, NGRP, AL)
            .transpose(2, 0, 1, 3).reshape(128, NCOLS // 32))
        eq = np.ascontiguousarray(
            edge[:, psl, :, 0].reshape(B, PLOC, NGRP, AL)
            .transpose(2, 0, 1, 3).reshape(128, NCOLS // 32))
        recvT = np.ascontiguousarray(
            reciever[:, psl, 0, :].reshape(NPAIR, F).T)
        in_maps.append({
            "expT": expT, "maskq": mq, "edgeq": eq,
            "recvT": recvT, "srcT": srcT, **shared,
        })
    return in_maps


def kernel(**inputs):
    if "nc" not in _CACHE:
        _CACHE["nc"] = _build_bass()
    nc = _CACHE["nc"]
    in_maps = _host_prep(inputs)

    from concourse.bass_utils import run_bass_kernel_spmd
    trace = os.environ.get("BASS_KERNEL_TRACE", "0") == "1"
    res = run_bass_kernel_spmd(nc, in_maps, core_ids=list(range(NCORES)),
                               trace=trace)
    _CACHE["last_result"] = res

    out = np.empty((B, P, G, F), np.float32)
    for c in range(NCORES):
        outT = res.results[c]["outT"]            # [F, NPAIR]
        out[:, c * PLOC : (c + 1) * PLOC, 0, :] = outT.T.reshape(B, PLOC, F)
    return out


# --- embedded TileContext fix (kernel.py must be self-contained) ---
import types as _types

_tile_fix_src = '''
import concourse.mybir as mybir
from concourse.tile import TileContext

MAX_WAITS = 1


def _split_instruction_waits(nc, drain_inst):
    si = drain_inst.ins.sync_info
    if si is None:
        return
    waits = list(si.on_wait)
    if len(waits) <= MAX_WAITS:
        return
    si.on_wait = waits[:MAX_WAITS]
    rest = waits[MAX_WAITS:]
    while rest:
        take, rest = rest[:MAX_WAITS], rest[MAX_WAITS:]
        d2 = nc.sync.drain()
        si2 = d2.ins.sync_info
        if si2 is None:
            d2.ins.sync_info = mybir.SyncInfo(on_wait=list(take), on_update=[])
        else:
            si2.on_wait = list(si2.on_wait) + list(take)


class SplitDrainTileContext(TileContext):
    def _drain_and_barrier(self, tick_clock, wait_clock):
        from concourse.vector_clock import ScopedClock

        drain_inst = self.nc.sync.drain()
        wait_clock.add_sem_waits(
            drain_inst.ins, ScopedClock({None: tick_clock.global_clock})
        )
        _split_instruction_waits(self.nc, drain_inst)

        self.nc.all_engine_barrier()
        assert self.sems is not None
        popped = self.nc._tile_sem_poison_stack.pop()
        assert popped is self._sem_poison
        self.nc.clear_and_free_semaphores(list(self.sems.allocated().values()))
        self.nc.all_engine_barrier()
'''


def _install_tile_fix():
    if "tile_fix_embedded" in sys.modules:
        return
    mod = _types.ModuleType("tile_fix_embedded")
    exec(_tile_fix_src, mod.__dict__)
    sys.modules["tile_fix_embedded"] = mod


_install_tile_fix()


# revision 32
# speedup vs baseline: 1.8604x; 1.4256x over previous
"""Trainium2 Bass kernel for DeepDFT Message+Receiver block.

Computes, for inputs of shape
  scalar [B,A,G,F], scalar_reciever [B,P,G,F], expansion [B,P,A,E],
  mask [B,P,A,G], edge_distance [B,P,A,1], + MLP weights:

  gates = ssp(expansion @ W1e + b1e) @ W2e + b2e
  gates *= 1 - sigmoid(5*(edge_distance - (CUTOFF-1.5)))
  src = scalar @ W1n[:F]; tgt = reciever @ W1n[F:]
  nodes = ssp(src + tgt + b1n) @ W2n + b2n
  out = sum_a mask * gates * nodes          -> [B,P,G,F]

Sharding: probe axis P across 8 cores. Within a core the atom axis is
split into 4 quarters mapped onto the 4 32-partition groups (features on
partitions), columns = (b, p, a_local).

Key identities (exact):
  ssp(x) = softplus(x) - log2 = ln(exp(x - log2) + 0.5)
  exp(src+tgt+b1n-log2) = exp(src+b1n-log2) * exp(tgt)   (tiny factors)
so each ssp costs one Ln pass on ACT; the gates path needs one extra Exp
pass; all biases fold into the exponent shifts (b1e/b1n/b2e/b2n are
handled generally below).
"""
import sys, os
if "/opt/trn_rl_repo" not in sys.path:
    sys.path.insert(0, "/opt/trn_rl_repo")
os.environ.setdefault("JAX_PLATFORMS", "cpu,axon")

import numpy as np
import ml_dtypes

B, P, A, G, F, E = 2, 4096, 96, 1, 32, 20
NCORES = 8
PLOC = P // NCORES          # 512 probes per core
NGRP = 4                    # atom quarter groups
AL = A // NGRP              # 24 atoms per group
NPAIR = B * PLOC            # 1024 (b,p) pairs per core
NCOLS = NPAIR * AL          # 24576 cols per group
CH = 512                    # chunk = one full psum bank = one j-block
NCH = NCOLS // CH           # 48 chunks
MACC = 3                    # chunks (j-blocks) per macro
NMAC = NCH // MACC          # 16 macros
LOG2 = 0.6931471805599453
CUTOFF = 5.0

_CACHE = {}

# Opcodes whose sem updates are executed by DMA hardware (riding the
# descriptor) rather than the issuing sequencer — their updates must not be
# moved onto a NOP.
_DMA_OPCODES = ("TensorLoad", "TensorSave", "TensorCopy", "Dge", "DMA")


def _fix_bir_json(raw: bytes) -> bytes:
    """This walrus build accepts at most ONE sem wait (and one update) per
    instruction (NEURON_ISA_TPB_EVENTS has a single wait/update slot).
    Split excess waits onto preceding same-engine NOPs (sequencer order
    makes this equivalent) and excess updates onto trailing NOPs."""
    import json
    m = json.loads(raw)
    ctr = [0]

    def mknop(engine, wait=None, upd=None):
        ctr[0] += 1
        return {
            "engine": engine, "ins": [], "outs": [],
            "name": f"I-wsplit-{ctr[0]}", "opcode": "NoOp",
            "sync_info": {
                "on_wait": [wait] if wait else [],
                "on_update": [upd] if upd else [],
            },
        }

    for fn in m["functions"]:
        for bb in fn["blocks"]:
            newl = []
            for inst in bb["instructions"]:
                si = inst.get("sync_info")
                pre, post = [], []
                if si:
                    w = si.get("on_wait") or []
                    if len(w) > 1:
                        for x in w[:-1]:
                            pre.append(mknop(inst["engine"], wait=x))
                        si["on_wait"] = [w[-1]]
                    u = si.get("on_update") or []
                    if len(u) > 1:
                        op = str(inst.get("opcode", ""))
                        assert not any(d in op for d in _DMA_OPCODES), (
                            f"multi-update DMA instruction {inst.get('name')}"
                        )
                        for x in u[1:]:
                            post.append(mknop(inst["engine"], upd=x))
                        si["on_update"] = [u[0]]
                newl.extend(pre)
                newl.append(inst)
                newl.extend(post)
            bb["instructions"] = newl
    return json.dumps(m).encode()


def _build_bass():
    import concourse.bass as bass
    import concourse.mybir as mybir
    from tile_fix_embedded import SplitDrainTileContext

    f32 = mybir.dt.float32
    bf16 = mybir.dt.bfloat16
    AF = mybir.ActivationFunctionType
    OP = mybir.AluOpType

    nc = bass.Bass(num_devices=NCORES)

    # ---- DRAM I/O ----
    d_expT = nc.dram_tensor("expT", [NGRP * E, NCOLS], bf16, kind="ExternalInput")
    d_maskq = nc.dram_tensor("maskq", [128, NCOLS // 32], f32, kind="ExternalInput")
    d_edgeq = nc.dram_tensor("edgeq", [128, NCOLS // 32], f32, kind="ExternalInput")
    d_recvT = nc.dram_tensor("recvT", [F, NPAIR], f32, kind="ExternalInput")
    d_srcT = nc.dram_tensor("srcT", [F, B * A], f32, kind="ExternalInput")
    d_bdW1e = nc.dram_tensor("bdW1e", [NGRP * E, 128], bf16, kind="ExternalInput")
    d_bdW2e = nc.dram_tensor("bdW2e", [128, 128], bf16, kind="ExternalInput")
    d_bdW2n = nc.dram_tensor("bdW2n", [128, 128], bf16, kind="ExternalInput")
    d_bdSum = nc.dram_tensor("bdSum", [128, F], f32, kind="ExternalInput")
    d_wsT = nc.dram_tensor("wsT", [F, F], f32, kind="ExternalInput")
    d_wtT = nc.dram_tensor("wtT", [F, F], f32, kind="ExternalInput")
    d_bEx = nc.dram_tensor("bEx", [128, 1], f32, kind="ExternalInput")    # b1e - log2 (x4)
    d_bEs = nc.dram_tensor("bEs", [F, 1], f32, kind="ExternalInput")      # b1n - log2
    # Ln scale/bias folds: act1 = Ln(E1*e^c + 0.5*e^c) = ssp(y1)+c, c = W2e^-T b2e
    d_lnSG = nc.dram_tensor("lnSG", [128, 1], f32, kind="ExternalInput")
    d_lnBG = nc.dram_tensor("lnBG", [128, 1], f32, kind="ExternalInput")
    d_lnBH = nc.dram_tensor("lnBH", [128, 1], f32, kind="ExternalInput")
    d_out = nc.dram_tensor("outT", [F, NPAIR], f32, kind="ExternalOutput")

    with SplitDrainTileContext(nc) as tc:
        with (
            tc.tile_pool(name="persist", bufs=1) as pp,
            tc.tile_pool(name="work", bufs=3) as wp,
            tc.tile_pool(name="stage", bufs=2) as sp,
            tc.tile_pool(name="psA", bufs=1, space="PSUM") as psA,
            tc.tile_pool(name="psC", bufs=2, space="PSUM") as psC,
        ):
            # ---- persistent tiles ----
            w1e = pp.tile([NGRP * E, 128], bf16, tag="w1e")
            w2e = pp.tile([128, 128], bf16, tag="w2e")
            w2n = pp.tile([128, 128], bf16, tag="w2n")
            wsum = pp.tile([128, F], f32, tag="wsum")
            ws = pp.tile([F, F], f32, tag="ws")
            wt = pp.tile([F, F], f32, tag="wt")
            bEx = pp.tile([128, 1], f32, tag="bEx")
            bEs = pp.tile([F, 1], f32, tag="bEs")
            lnSG = pp.tile([128, 1], f32, tag="lnSG")
            lnBG = pp.tile([128, 1], f32, tag="lnBG")
            lnBH = pp.tile([128, 1], f32, tag="lnBH")
            nc.sync.dma_start(out=w1e[:], in_=d_bdW1e[:])
            nc.sync.dma_start(out=w2e[:], in_=d_bdW2e[:])
            nc.sync.dma_start(out=w2n[:], in_=d_bdW2n[:])
            nc.sync.dma_start(out=wsum[:], in_=d_bdSum[:])
            nc.sync.dma_start(out=ws[:], in_=d_wsT[:])
            nc.sync.dma_start(out=wt[:], in_=d_wtT[:])
            nc.sync.dma_start(out=bEx[:], in_=d_bEx[:])
            nc.sync.dma_start(out=bEs[:], in_=d_bEs[:])
            nc.sync.dma_start(out=lnSG[:], in_=d_lnSG[:])
            nc.sync.dma_start(out=lnBG[:], in_=d_lnBG[:])
            nc.sync.dma_start(out=lnBH[:], in_=d_lnBH[:])

            # ---- s = mask * sigmoid(17.5 - 5 d): do Sigmoid FIRST (table set) ----
            mq = pp.tile([128, NCOLS // 32], f32, tag="mq")
            eq = pp.tile([128, NCOLS // 32], f32, tag="eq")
            nc.sync.dma_start(out=mq[:], in_=d_maskq[:])
            nc.sync.dma_start(out=eq[:], in_=d_edgeq[:])
            sigB = pp.tile([128, 1], f32, tag="sigB")
            nc.gpsimd.memset(sigB[:], 5.0 * (CUTOFF - 1.5))
            sig = pp.tile([128, NCOLS // 32], f32, tag="sig")
            nc.scalar.activation(sig[:], eq[:], AF.Sigmoid,
                                 bias=sigB[:, 0:1], scale=-5.0)
            sqb = pp.tile([128, NCOLS // 32], bf16, tag="sqb")
            nc.vector.tensor_mul(out=sqb[:], in0=mq[:], in1=sig[:])

            # ---- s to DRAM, replicated to all 32 feature partitions of each
            # group via 5 DRAM->DRAM log-doublings; per-macro slices then load
            # with a plain strided AP.
            d_sb32 = nc.dram_tensor("sb32", [NGRP, 32, NCOLS], bf16)
            nc.gpsimd.dma_start(out=d_sb32[:, 0:1, :], in_=sqb[:])
            rep = 1
            while rep < 32:
                nc.gpsimd.dma_start(
                    out=d_sb32[:, rep : 2 * rep, :], in_=d_sb32[:, 0:rep, :])
                rep *= 2

            # ---- es4 [128, B*AL], et4 [128, NPAIR] (bf16, exp domain) ----
            srcT = pp.tile([F, B * A], f32, tag="srcT")
            recvT = pp.tile([F, NPAIR], f32, tag="recvT")
            nc.sync.dma_start(out=srcT[:], in_=d_srcT[:])
            nc.sync.dma_start(out=recvT[:], in_=d_recvT[:])

            ps_s = psC.tile([F, B * A], f32, tag="psG")
            nc.tensor.matmul(ps_s[:], ws[:], srcT[:], start=True, stop=True)
            es_full = pp.tile([F, B * A], f32, tag="es_full")
            nc.scalar.activation(es_full[:], ps_s[:], AF.Exp, bias=bEs[:, 0:1])

            et_full = pp.tile([F, NPAIR], bf16, tag="et_full")
            for half in range(2):
                ps_t = psC.tile([F, 512], f32, tag="psN")
                nc.tensor.matmul(ps_t[:], wt[:], recvT[:, 512 * half : 512 * (half + 1)],
                                 start=True, stop=True)
                nc.scalar.activation(et_full[:, 512 * half : 512 * (half + 1)],
                                     ps_t[:], AF.Exp)

            es4 = pp.tile([128, B * AL], f32, tag="es4")
            et4 = pp.tile([128, NPAIR], bf16, tag="et4")
            for i in range(NGRP):
                for b in range(B):
                    nc.sync.dma_start(
                        out=es4[32 * i : 32 * i + 32, b * AL : (b + 1) * AL],
                        in_=es_full[:, b * A + AL * i : b * A + AL * (i + 1)],
                    )
                nc.sync.dma_start(out=et4[32 * i : 32 * i + 32, :], in_=et_full[:])

            # ---- output accumulator ----
            OUT4 = pp.tile([128, NPAIR], f32, tag="OUT4")

            # ---- main loop (column order per group: b, j=a_local, p) ----
            MW = MACC * CH        # 1536 = 3 j-blocks of 512 probes
            MPB = NMAC // B       # macros per b
            sb32f = d_sb32[:].rearrange("i k m -> (i k) m")
            for m in range(NMAC):
                bidx = m // MPB
                j0 = MACC * (m % MPB)
                if m % MPB == 0:
                    stage = sp.tile([128, AL * PLOC], bf16, tag="stage")

                X = wp.tile([NGRP * E, MW], bf16, tag="X")
                nc.sync.dma_start(out=X[:], in_=d_expT[:, m * MW : (m + 1) * MW])

                Sl = wp.tile([128, MW], bf16, tag="Sl")
                nc.gpsimd.dma_start(out=Sl[:],
                                    in_=sb32f[:, m * MW : (m + 1) * MW])

                ps1 = psA.tile([128, MW], f32, tag="ps1")
                for c in range(MACC):
                    nc.tensor.matmul(
                        ps1[:, CH * c : CH * (c + 1)],
                        w1e[:], X[:, CH * c : CH * (c + 1)],
                        start=True, stop=True)
                E1 = wp.tile([128, MW], f32, tag="E1")
                nc.scalar.activation(E1[:], ps1[:], AF.Exp, bias=bEx[:, 0:1])

                act1 = wp.tile([128, MW], bf16, tag="act1")
                nc.scalar.activation(act1[:], E1[:], AF.Ln,
                                     bias=lnBG[:, 0:1], scale=lnSG[:, 0:1])

                # actH = Ln(es[b,j] * et[b,p] + bias): es is constant per
                # j-block, so it rides the ACT scale vector — no DVE product.
                actH = wp.tile([128, MW], bf16, tag="actH")
                for c in range(MACC):
                    j = j0 + c
                    nc.scalar.activation(
                        actH[:, CH * c : CH * (c + 1)],
                        et4[:, bidx * PLOC : (bidx + 1) * PLOC],
                        AF.Ln, bias=lnBH[:, 0:1],
                        scale=es4[:, bidx * AL + j : bidx * AL + j + 1])

                # per-chunk mm2/mmH with 1-bank double-buffered psum
                for c in range(MACC):
                    psG = psC.tile([128, CH], f32, tag="psG")
                    nc.tensor.matmul(
                        psG[:], w2e[:], act1[:, CH * c : CH * (c + 1)],
                        start=True, stop=True)
                    psN = psC.tile([128, CH], f32, tag="psN")
                    nc.tensor.matmul(
                        psN[:], w2n[:], actH[:, CH * c : CH * (c + 1)],
                        start=True, stop=True)
                    # sq = (G * s) * N — DVE reads one PSUM input per op
                    gs = wp.tile([128, CH], bf16, tag="q")
                    nc.vector.tensor_mul(
                        out=gs[:], in0=psG[:], in1=Sl[:, c * CH : (c + 1) * CH])
                    nc.vector.tensor_mul(
                        out=stage[:, (j0 + c) * CH : (j0 + c + 1) * CH],
                        in0=gs[:], in1=psN[:])

                if m % MPB == MPB - 1:
                    # tree-reduce stage [128, (j=24, p=512)] over j, in two
                    # p-halves for finer overlap
                    HP = PLOC // 2
                    for h2 in range(2):
                        sv = stage[:].rearrange("p (j q) -> p j q", j=AL)[
                            :, :, h2 * HP : (h2 + 1) * HP]
                        t1 = sp.tile([128, 12 * HP], bf16, tag="t1")
                        t1v = t1[:].rearrange("p (j q) -> p j q", j=12)
                        nc.vector.tensor_add(out=t1v, in0=sv[:, 0:12, :],
                                             in1=sv[:, 12:24, :])
                        t2 = sp.tile([128, 6 * HP], bf16, tag="t2")
                        t2v = t2[:].rearrange("p (j q) -> p j q", j=6)
                        nc.vector.tensor_add(out=t2v, in0=t1v[:, 0:6, :],
                                             in1=t1v[:, 6:12, :])
                        t3 = sp.tile([128, 3 * HP], bf16, tag="t3")
                        t3v = t3[:].rearrange("p (j q) -> p j q", j=3)
                        nc.vector.tensor_add(out=t3v, in0=t2v[:, 0:3, :],
                                             in1=t2v[:, 3:6, :])
                        t4 = sp.tile([128, HP], f32, tag="t4")
                        nc.vector.tensor_add(
                            out=t4[:, None, :], in0=t3v[:, 0:1, :],
                            in1=t3v[:, 1:2, :])
                        ovs = OUT4[:, bidx * PLOC + h2 * HP :
                                   bidx * PLOC + (h2 + 1) * HP]
                        nc.vector.tensor_add(
                            out=ovs[:, None, :], in0=t4[:, None, :],
                            in1=t3v[:, 2:3, :])

            # ---- cross-group sum + writeout ----
            outsb = pp.tile([F, NPAIR], f32, tag="outsb")
            for half in range(2):
                psF = psC.tile([F, 512], f32, tag="psN")
                nc.tensor.matmul(psF[:], wsum[:],
                                 OUT4[:, 512 * half : 512 * (half + 1)],
                                 start=True, stop=True)
                nc.vector.tensor_copy(outsb[:, 512 * half : 512 * (half + 1)], psF[:])
            nc.sync.dma_start(out=d_out[:], in_=outsb[:])

    # Patch serialization: enforce the 1-wait/1-update ISA slot limit.
    import types
    _orig_tjb = nc.to_json_bytes
    _fixed = {}

    def _patched_to_json_bytes(self):
        if "b" not in _fixed:
            _fixed["b"] = _fix_bir_json(_orig_tjb())
        return _fixed["b"]

    nc.to_json_bytes = types.MethodType(_patched_to_json_bytes, nc)
    return nc


def _host_prep(inputs):
    """Host-side layout prep: slicing/transpose/padding only (plus constant
    folds on the tiny weight matrices)."""
    scalar = np.asarray(inputs["scalar"], np.float32)
    reciever = np.asarray(inputs["scalar_reciever"], np.float32)
    expansion = np.asarray(inputs["expansion"], np.float32)
    mask = np.asarray(inputs["mask"], np.float32)
    edge = np.asarray(inputs["edge_distance"], np.float32)
    W1e = np.asarray(inputs["W1e"], np.float32)
    b1e = np.asarray(inputs["b1e"], np.float32)
    W2e = np.asarray(inputs["W2e"], np.float32)
    b2e = np.asarray(inputs["b2e"], np.float32)
    W1n = np.asarray(inputs["W1n"], np.float32)
    b1n = np.asarray(inputs["b1n"], np.float32)
    W2n = np.asarray(inputs["W2n"], np.float32)
    b2n = np.asarray(inputs["b2n"], np.float32)

    bdW1e = np.zeros((NGRP * E, 128), np.float32)
    bdW2e = np.zeros((128, 128), np.float32)
    bdW2n = np.zeros((128, 128), np.float32)
    bdSum = np.zeros((128, F), np.float32)
    for i in range(NGRP):
        bdW1e[i * E : (i + 1) * E, 32 * i : 32 * i + F] = W1e
        bdW2e[32 * i : 32 * i + F, 32 * i : 32 * i + F] = W2e
        bdW2n[32 * i : 32 * i + F, 32 * i : 32 * i + F] = W2n
        bdSum[32 * i : 32 * i + F, :] = np.eye(F, dtype=np.float32)
    # act1 = Ln(E1*e^cg + 0.5*e^cg) = ssp(y1) + cg with cg = W2e^-T b2e, so
    # act1 @ W2e = ssp @ W2e + b2e exactly (same for the nodes path).
    cg = np.linalg.solve(W2e.T.astype(np.float64), b2e.astype(np.float64))
    cn = np.linalg.solve(W2n.T.astype(np.float64), b2n.astype(np.float64))
    bf = ml_dtypes.bfloat16
    shared = {
        "bdW1e": bdW1e.astype(bf), "bdW2e": bdW2e.astype(bf),
        "bdW2n": bdW2n.astype(bf), "bdSum": bdSum,
        "wsT": np.ascontiguousarray(W1n[:F]),
        "wtT": np.ascontiguousarray(W1n[F:]),
        "bEx": np.ascontiguousarray((np.tile(b1e, NGRP) - LOG2)[:, None]),
        "bEs": np.ascontiguousarray((b1n - LOG2 + cn.astype(np.float32))[:, None]).astype(np.float32),
        "lnSG": np.tile(np.exp(cg), NGRP).astype(np.float32)[:, None].copy(),
        "lnBG": np.tile(0.5 * np.exp(cg), NGRP).astype(np.float32)[:, None].copy(),
        "lnBH": np.tile(0.5 * np.exp(cn), NGRP).astype(np.float32)[:, None].copy(),
    }
    srcT = np.ascontiguousarray(scalar[:, :, 0, :].reshape(B * A, F).T)

    in_maps = []
    for c in range(NCORES):
        psl = slice(c * PLOC, (c + 1) * PLOC)
        x = expansion[:, psl].reshape(B, PLOC, NGRP, AL, E)
        expT = np.ascontiguousarray(
            x.transpose(2, 4, 0, 3, 1).reshape(NGRP * E, NCOLS)).astype(
                ml_dtypes.bfloat16)
        mq = np.ascontiguousarray(
            mask[:, psl, :, 0].reshape(B, PLOC, NGRP, AL)
            .transpose(2, 0, 3, 1).reshape(128, NCOLS // 32))
        eq = np.ascontiguousarray(
            edge[:, psl, :, 0].reshape(B, PLOC, NGRP, AL)
            .transpose(2, 0, 3, 1).reshape(128, NCOLS // 32))
        recvT = np.ascontiguousarray(
            reciever[:, psl, 0, :].reshape(NPAIR, F).T)
        in_maps.append({
            "expT": expT, "maskq": mq, "edgeq": eq,
            "recvT": recvT, "srcT": srcT, **shared,
        })
    return in_maps


def kernel(**inputs):
    if "nc" not in _CACHE:
        _CACHE["nc"] = _build_bass()
    nc = _CACHE["nc"]
    in_maps = _host_prep(inputs)

    from concourse.bass_utils import run_bass_kernel_spmd
    trace = os.environ.get("BASS_KERNEL_TRACE", "0") == "1"
    res = run_bass_kernel_spmd(nc, in_maps, core_ids=list(range(NCORES)),
                               trace=trace)
    _CACHE["last_result"] = res

    out = np.empty((B, P, G, F), np.float32)
    for c in range(NCORES):
        outT = res.results[c]["outT"]            # [F, NPAIR]
        out[:, c * PLOC : (c + 1) * PLOC, 0, :] = outT.T.reshape(B, PLOC, F)
    return out


# --- embedded TileContext fix (kernel.py must be self-contained) ---
import types as _types

_tile_fix_src = '''
import concourse.mybir as mybir
from concourse.tile import TileContext

MAX_WAITS = 1


def _split_instruction_waits(nc, drain_inst):
    si = drain_inst.ins.sync_info
    if si is None:
        return
    waits = list(si.on_wait)
    if len(waits) <= MAX_WAITS:
        return
    si.on_wait = waits[:MAX_WAITS]
    rest = waits[MAX_WAITS:]
    while rest:
        take, rest = rest[:MAX_WAITS], rest[MAX_WAITS:]
        d2 = nc.sync.drain()
        si2 = d2.ins.sync_info
        if si2 is None:
            d2.ins.sync_info = mybir.SyncInfo(on_wait=list(take), on_update=[])
        else:
            si2.on_wait = list(si2.on_wait) + list(take)


class SplitDrainTileContext(TileContext):
    def _drain_and_barrier(self, tick_clock, wait_clock):
        from concourse.vector_clock import ScopedClock

        drain_inst = self.nc.sync.drain()
        wait_clock.add_sem_waits(
            drain_inst.ins, ScopedClock({None: tick_clock.global_clock})
        )
        _split_instruction_waits(self.nc, drain_inst)

        self.nc.all_engine_barrier()
        assert self.sems is not None
        popped = self.nc._tile_sem_poison_stack.pop()
        assert popped is self._sem_poison
        self.nc.clear_and_free_semaphores(list(self.sems.allocated().values()))
        self.nc.all_engine_barrier()
'''


def _install_tile_fix():
    if "tile_fix_embedded" in sys.modules:
        return
    mod = _types.ModuleType("tile_fix_embedded")
    exec(_tile_fix_src, mod.__dict__)
    sys.modules["tile_fix_embedded"] = mod


_install_tile_fix()
